# revision 57
# baseline (speedup 1.0000x reference)
"""Trainium2 Bass kernel for AgentCapabilityEstimator (dense MLP, 3 heads).

Reference computation (B=16384, OBS=512, H=1024, N=9):
    g  = relu(relu(obs @ W1 + b1) @ W2 + b2)                    [B, H]
    cov  = sigmoid(relu(g @ Wc1 + bc1) @ Wc2 + bc2)             [B, 1]
    trk  = sigmoid(relu(g @ Wt1 + bt1) @ Wt2 + bt2)             [B, 1]
    coop = sigmoid(relu([g,g] @ Wk1 + bk1) @ Wk2 + bk2)         [B, 1]
    outputs broadcast to [B, 9] each.

Strategy: pure data parallelism over 8 cores (2048 rows each), with all four
GEMM stages in fp8(e4m3) DoubleRow mode (256-deep contraction per matmul).
Each matmul streams the full 512-column batch tile through one stationary
load, so the PE's weight-load pipe (the 131ns/instr cadence limiter at
256-column moving) is half as loaded and the instruction count halves.
Weights are scaled by 64 host-side so they sit in e4m3's normal range; each
layer's activation applies a compile-time descale and a power-of-two
activation quantisation scale before casting back to fp8.  The final sigmoid
applies the exact inverse scale, so only quantisation noise (<1.1e-2 on this
model, threshold 2e-2) remains.  Relu/quant work is split across the scalar
and vector engines so neither blocks the tensor engine.  Host-side prep folds
Wk1 ([g,g] @ Wk1 == g @ (Wk1_hi + Wk1_lo)), concatenates the three head
hidden layers into one [1024, 2048] GEMM, and packs the three scalar head
outputs into one [2048, 3] block-sparse final matmul.  The device emits only
the compact [3, 512] sigmoid block per tile (3 x 2KB DMA descriptors); the
[B, 9] broadcast happens on the host, eliminating the on-device transpose
matmuls and the 36-byte-descriptor output DMA drain of the previous version.
"""

import numpy as np
import ml_dtypes

import concourse.bass as bass
import concourse.mybir as mybir
import concourse.tile as tile
from concourse import bacc
from concourse.bass_utils import run_bass_kernel_spmd

B, OBS, H, N = 16384, 512, 1024, 9
NCORES = 8
BC = B // NCORES          # 2048 batch rows per core
NTILE = 512               # batch rows per compute tile
NT = BC // NTILE          # 4 tiles per core
P = 128
KO = OBS // P             # 4 obs chunks
HO = H // P               # 8 hidden chunks
AO = 2 * H // P           # 16 chunks of the stacked head-hidden features
FINW = 32                 # final-matmul stationary width (3 live + zero pad;
                          # DoubleRow Ldweights ISA requires a full 32-wide tile)
F32 = mybir.dt.float32
F8 = mybir.dt.float8e4
NP_F8 = ml_dtypes.float8_e4m3

# fp8 scaling: weights x64 (e4m3 normal range), activations quantised at
# power-of-two scales S; descale constants are exact in fp32.
WS = 64.0
S1, S2, S3 = 4.0, 8.0, 16.0
DS1 = S1 / WS             # psum1 * DS1 = S1 * z1
DS2 = S2 / (WS * S1)
DS3 = S3 / (WS * S2)
DSF = 1.0 / (WS * S3)     # final psum * DSF = true logit

RELU = mybir.ActivationFunctionType.Relu
SIGMOID = mybir.ActivationFunctionType.Sigmoid
DR = mybir.MatmulPerfMode.DoubleRow
MULT = mybir.AluOpType.mult
MAX = mybir.AluOpType.max


def build_nc(zero_bias: bool) -> bass.Bass:
    nc = bacc.Bacc(trn_type="TRN2", target_bir_lowering=False, debug=False)

    # All tensors are pre-arranged host-side to partition-major layouts so
    # every DMA is 128 large contiguous runs (one per partition) instead of
    # thousands of sub-KB descriptors on one hardware queue.  Input DMAs are
    # split across both hardware DGE queues: obs/W1 on the SP queue (first
    # matmul's critical path), later-phase weights on the Activation queue.
    obsA = nc.dram_tensor("obsA", [P, NT * KO * NTILE], F8,
                          kind="ExternalInput").ap()
    W1 = nc.dram_tensor("W1A", [P, KO * H], F8, kind="ExternalInput").ap()
    W2 = nc.dram_tensor("W2A", [P, HO * H], F8, kind="ExternalInput").ap()
    Wh = nc.dram_tensor("WhA", [P, HO * 2 * H], F8, kind="ExternalInput").ap()
    Wfin = nc.dram_tensor("WfinA", [P, AO * FINW], F8,
                          kind="ExternalInput").ap()
    # all four bias vectors packed into one DMA: cols 0:8 b1, 8:16 b2,
    # 16:32 bh, col 32 rows 0:3 bfin
    ball = nc.dram_tensor("ballA", [P, 2 * HO + AO + 1], F32,
                          kind="ExternalInput").ap()
    out_sig = nc.dram_tensor("sig", [3, NT * NTILE], F32,
                             kind="ExternalOutput").ap()

    with tile.TileContext(nc) as tc:
        _body(tc, zero_bias, obsA, W1, W2, Wh, Wfin, ball, out_sig)
    nc.compile()
    return nc


def _body(tc, zero_bias, obsA, W1, W2, Wh, Wfin, ball, out_sig):
    nc = tc.nc

    with (
        tc.tile_pool(name="weights", bufs=1) as wpool,
        tc.tile_pool(name="obs", bufs=1) as obspool,
        tc.tile_pool(name="acts", bufs=3) as actpool,
        tc.tile_pool(name="hpool", bufs=3) as hpool,
        tc.tile_pool(name="gpool", bufs=3) as gpool,
        tc.tile_pool(name="small", bufs=2) as smallpool,
        tc.tile_pool(name="psum", bufs=7, space="PSUM") as psum,
        tc.tile_pool(name="psum_fin", bufs=1, space="PSUM") as psum_f,
    ):
        # ---- resident weights / biases ----------------------------------
        # W1/W2 are laid out m-block-major (host side) so the first output
        # blocks' weights can land -- and unblock the first groups -- before
        # the rest of the tensor arrives.
        obsA_r = obsA.rearrange("p (t k j) -> p t k j", t=NT, k=KO)
        w1_sb = wpool.tile([P, 4, KO, 2 * P], F8)     # quarter = 2 m-blocks
        w2_sb = wpool.tile([P, 4, HO, 2 * P], F8)     # quarter = 2 m-blocks
        wh_sb = wpool.tile([P, HO, 2 * H], F8)

        # The ~16 DMA channels drain both DGE queues' descriptor streams
        # round-robin at ~300GB/s aggregate, so a transfer completes by its
        # cumulative position in the combined stream — and every dma_start
        # costs 0.6-1.3us of *issue* time on its sequencer.  Strict
        # need-order, few transfers: the first real matmul's data (W1 + obs
        # tile 0, 768KB) completes ~2.6us after DMA-go, everything else
        # lands a phase or more before its first consumer.  The Act queue
        # carries no further issues after W2 so the scalar engine is free
        # for L1 relu/quants from ~11.5us on.
        x_all = obspool.tile([P, NT, KO, NTILE], F8)
        xs = {t: x_all[:, t] for t in range(NT)}
        wfin_sb = wpool.tile([P, AO, FINW], F8)
        ball_sb = wpool.tile([P, 2 * HO + AO + 1], F32)
        b1_sb = ball_sb[:, 0:HO]
        b2_sb = ball_sb[:, HO:2 * HO]
        bh_sb = ball_sb[:, 2 * HO:2 * HO + AO]
        bfin_sb = ball_sb[0:3, 2 * HO + AO:]
        # The two DGE queues drain concurrently at ~110GB/s each while both
        # are loaded, and a transfer's completion SEMAPHORE fires ~1.3us
        # after its last data descriptor.  Everything is ordered strictly by
        # first-consumer time, with the critical tensors split into blocks
        # across both queues: obs tile 0 + W1 quarters gate the first real
        # matmuls, ball gates the first scalar-engine relu, W2 quarters gate
        # the L2 groups of the first zipped phase.
        W1r = W1.rearrange("p (q c h) -> p q c h", q=4, c=KO)
        W2r = W2.rearrange("p (q c h) -> p q c h", q=4, c=HO)
        x0 = xs[0]
        x1 = xs[1]
        nc.sync.dma_start(out=x0[:, 0:2], in_=obsA_r[:, 0, 0:2, :])
        nc.sync.dma_start(out=w1_sb[:, 0], in_=W1r[:, 0])
        nc.sync.dma_start(out=w1_sb[:, 2], in_=W1r[:, 2])
        nc.sync.dma_start(out=x1[:, 0:2], in_=obsA_r[:, 1, 0:2, :])
        nc.sync.dma_start(out=w2_sb[:, 1], in_=W2r[:, 1])
        nc.sync.dma_start(out=w2_sb[:, 3], in_=W2r[:, 3])
        nc.sync.dma_start(out=x_all[:, 2], in_=obsA_r[:, 2, :, :])
        nc.sync.dma_start(out=wfin_sb,
                          in_=Wfin.rearrange("p (c m) -> p c m", c=AO))
        nc.sync.dma_start(out=wh_sb, in_=Wh.rearrange("p (c h) -> p c h", c=HO))
        nc.scalar.dma_start(out=ball_sb[:, 0:2 * HO], in_=ball[:, 0:2 * HO])
        nc.scalar.dma_start(out=x0[:, 2:4], in_=obsA_r[:, 0, 2:4, :])
        nc.scalar.dma_start(out=w1_sb[:, 1], in_=W1r[:, 1])
        nc.scalar.dma_start(out=w1_sb[:, 3], in_=W1r[:, 3])
        nc.scalar.dma_start(out=x1[:, 2:4], in_=obsA_r[:, 1, 2:4, :])
        nc.scalar.dma_start(out=w2_sb[:, 0], in_=W2r[:, 0])
        nc.scalar.dma_start(out=w2_sb[:, 2], in_=W2r[:, 2])
        nc.scalar.dma_start(out=ball_sb[:, 2 * HO:], in_=ball[:, 2 * HO:])
        nc.scalar.dma_start(out=x_all[:, 3], in_=obsA_r[:, 3, :, :])

        # PE warm-up: the tensor engine ramps to full clock only after ~3us
        # of continuous execution; without this, the first ~20 real matmuls
        # run at half rate while also waiting on the W1/x0 DMAs.  Burn the
        # DMA-wait window with dummy DoubleRow matmuls on a zeroed tile.
        # The memset runs on GpSimd (idle at program start) so the PE does
        # not wait on the DVE/Act engines' first-instruction latency.
        warm = obspool.tile([P, 2, 256], F8, tag="warm")
        nc.gpsimd.memset(warm, 0)
        # scratch PSUM for warm-up matmuls lives in the fin pool: fins are
        # first used phases later, so there is no rotation conflict and no
        # interleaved accumulation group sharing its zero region.
        wps = psum_f.tile([P, 256], F32, tag="fin", name="warmps")

        def warm_mm(n):
            for _ in range(n):
                nc.tensor.matmul(wps, warm[:, :, 0:P], warm,
                                 start=True, stop=True, perf_mode=DR)

        def warm_b():
            """One dependency-free matmul at a phase boundary: it issues
            straight after the previous phase's last matmul, so the real
            first matmul's semaphore check and LdWeights stream during it
            instead of draining the weight pipe (~160ns saved per phase).
            Uses the rotating mm pool (the fin bank belongs to ps3 here)."""
            ps = psum.tile([P, 256], F32, tag="mm", name="warmb")
            nc.tensor.matmul(ps, warm[:, :, 0:P], warm,
                             start=True, stop=True, perf_mode=DR)

        # warms (mid p-state, ~213ns each) burn the DMA-wait window; sized
        # to end just before the critical obs/W1-block semaphores fire.
        warm_mm(20)

        def act_relu(out, ps, bias_sb, ds, use_dve):
            """out_fp8 = S*relu(z+b): scalar path relu(ps*ds + S*b),
            DVE path (zero bias only) max(ps*ds, 0).  GpSimd/Pool cannot
            read PSUM on TRN2, so only these two engines split the acts."""
            if use_dve and zero_bias:
                nc.vector.tensor_scalar(out, ps, ds, 0.0, MULT, MAX)
            else:
                nc.scalar.activation(out, ps, RELU, bias=bias_sb, scale=ds)

        g1s = {}
        gs = {}

        def group(x, wap, b_sb, ds, out, m, kchunks, dve):
            """one m-chunk: 256-deep DoubleRow matmuls streaming the full
            512-column batch tile per stationary load + relu/quant"""
            ps = psum.tile([P, NTILE], F32, tag="mm")
            for k in range(kchunks // 2):
                nc.tensor.matmul(
                    ps, wap(m, k), x[:, 2 * k:2 * k + 2, :],
                    start=(k == 0), stop=(k == kchunks // 2 - 1),
                    perf_mode=DR)
            act_relu(out[:, m, :], ps, b_sb[:, m:m + 1], ds, dve)

        def w1ap(m, k):
            return w1_sb[:, m // 2, 2 * k:2 * k + 2,
                         (m % 2) * P:(m % 2 + 1) * P]

        def w2ap(m, k):
            return w2_sb[:, m // 2, 2 * k:2 * k + 2,
                         (m % 2) * P:(m % 2 + 1) * P]

        def l1(t):
            # Only used for the first tile (later L1s are zipped with an L2).
            # Runs while the PE clock is still ramping (~850ns/group at mid
            # p-state), which the alternating act engines (~690ns each)
            # absorb without padding.
            x = xs.pop(t)
            g1 = g1s[t] = actpool.tile([P, HO, NTILE], F8, tag="g1",
                           name=f"g1_{t}")
            for m in range(HO):
                group(x, w1ap, b1_sb, DS1, g1, m, KO, m % 2 == 1)

        def l1_l2(ta, tb, front=3):
            """L1(ta) and L2(tb) zipped at group granularity: an L1 group
            alone (~427ns of PE) outruns its relu/quant (~690ns), so pair
            each with an L2 group (~854ns) to keep both act engines fed
            without stalling the PE on PSUM recycle.  The first L2 group
            needs ALL EIGHT of tile tb's L1 relu/quants, and the last of
            those completes ~1.4us after the previous phase's final matmul;
            front-loading `front` extra L1 groups gives that act chain (and,
            for the first zipped phase, the W2-quarter DMAs) time to land so
            L2(tb, m=0) never stalls the PE."""
            warm_b()
            x = xs.pop(ta)
            g1 = g1s[ta] = actpool.tile([P, HO, NTILE], F8, tag="g1",
                            name=f"g1_{ta}")
            g1b = g1s.pop(tb)
            g = gs[tb] = gpool.tile([P, HO, NTILE], F8, tag="g",
                        name=f"g_{tb}")
            for m in range(HO + front):
                if m < HO:
                    group(x, w1ap, b1_sb, DS1, g1, m, KO, m % 2 == 1)
                if m >= front:
                    group(g1b, w2ap, b2_sb, DS2, g, m - front, HO,
                          m % 2 == 0)

        def l2(t):
            warm_b()
            g1 = g1s.pop(t)
            g = gs[t] = gpool.tile([P, HO, NTILE], F8, tag="g",
                       name=f"g_{t}")
            for m in range(HO):
                group(g1, w2ap, b2_sb, DS2, g, m, HO, m % 2 == 1)

        def heads(t):
            warm_b()
            g = gs.pop(t)
            # head hiddens h = S3*relu(Wh.T @ g + bh), produced in chunk
            # pairs; each pair feeds one DoubleRow final matmul, emitted one
            # pair LATE so the pair's relu/quant (on scalar/DVE) hides behind
            # the next pair's 4 matmuls instead of stalling the PE.  The fin
            # accumulation is one sequential group over j in a single PSUM
            # bank (512-column moving), so no bank interleaving hazards.
            ps3 = psum_f.tile([FINW, NTILE], F32, tag="fin", name=f"fin{t}")
            h2s = {}

            def fin(j):
                nc.tensor.matmul(
                    ps3,
                    wfin_sb[:, 2 * j:2 * j + 2, :],
                    h2s[j],
                    start=(j == 0), stop=(j == AO // 2 - 1),
                    perf_mode=DR)

            for m in range(AO):
                j = m // 2
                if m % 2 == 0:
                    h2s[j] = hpool.tile([P, 2, NTILE], F8, tag="h",
                                        name=f"h{t}_{j}")
                ps = psum.tile([P, NTILE], F32, tag="mm")
                for k in range(HO // 2):
                    nc.tensor.matmul(
                        ps,
                        wh_sb[:, 2 * k:2 * k + 2, m * P:(m + 1) * P],
                        g[:, 2 * k:2 * k + 2, :],
                        start=(k == 0), stop=(k == HO // 2 - 1),
                        perf_mode=DR)
                # parity chosen so the LAST pair's act (which gates the
                # final fin + output tail) lands on the faster scalar path
                act_relu(h2s[j][:, m % 2, :], ps, bh_sb[:, m:m + 1], DS3,
                         m % 2 == 0)
                if m % 2 == 1 and j >= 1:
                    fin(j - 1)
                    h2s.pop(j - 1)
            fin(AO // 2 - 1)
            # the raw fin logits only need PSUM->SBUF before the output DMA;
            # the host applies scale + bias + sigmoid (identical f32 math).
            # Halved across the scalar and vector engines so the tail-gating
            # copy costs ~0.4us instead of ~0.7us.
            hf = NTILE // 2
            sig = smallpool.tile([3, NTILE], F32, tag="sig")
            nc.scalar.activation(sig[:, 0:hf], ps3[0:3, 0:hf],
                                 mybir.ActivationFunctionType.Copy)
            nc.vector.tensor_copy(out=sig[:, hf:], in_=ps3[0:3, hf:])
            nc.sync.dma_start(
                out=out_sig.rearrange("m (t j) -> m t j", t=NT)[:, t, :],
                in_=sig)

        # Software-pipelined phase order: every layer phase is separated from
        # its producer phase by at least one unrelated phase, so the
        # producer's last relu/quant (scalar/DVE) lands well before the
        # consumer's PE matmuls need it — no layer-boundary PE stalls.
        l1(0)
        l1_l2(1, 0, front=5)
        l1_l2(2, 1)
        heads(0)
        l1_l2(3, 2)
        heads(1)
        l2(3)
        heads(2)
        heads(3)


_NC_CACHE = {}


def _get_nc(zero_bias: bool = True) -> bass.Bass:
    if zero_bias not in _NC_CACHE:
        _NC_CACHE[zero_bias] = build_nc(zero_bias)
    return _NC_CACHE[zero_bias]


def prep_inputs(obs, W1, b1, W2, b2, Wc1, bc1, Wc2, bc2,
                Wt1, bt1, Wt2, bt2, Wk1, bk1, Wk2, bk2, **_unused):
    """Host-side prep: fold/concat weights, scale + quantise to fp8,
    transpose obs, build shards."""
    f = np.float32

    def q8(a):
        return np.ascontiguousarray(
            np.clip(np.asarray(a, f), -240.0, 240.0).astype(NP_F8))

    Wk1f = np.asarray(Wk1[:H], f) + np.asarray(Wk1[H:], f)     # [H, H]
    Wh = np.concatenate([np.asarray(Wc1, f), np.asarray(Wt1, f), Wk1f],
                        axis=1)                                # [H, 2H]
    Wfin = np.zeros((2 * H, FINW), f)
    Wfin[0:H // 2, 0] = np.asarray(Wc2, f)[:, 0]
    Wfin[H // 2:H, 1] = np.asarray(Wt2, f)[:, 0]
    Wfin[H:2 * H, 2] = np.asarray(Wk2, f)[:, 0]
    bhc = np.concatenate([np.asarray(bc1, f), np.asarray(bt1, f),
                          np.asarray(bk1, f)])                 # [2H]
    bfin = np.array([np.asarray(bc2, f)[0], np.asarray(bt2, f)[0],
                     np.asarray(bk2, f)[0]], f)

    # biases folded at each layer's activation quantisation scale,
    # pre-arranged to [partition, chunk]
    def barr(bq, chunks):
        return np.ascontiguousarray(
            bq.reshape(chunks, P).T.astype(f))           # [P, chunks]

    b1q = (S1 * np.asarray(b1, f)).astype(f)
    b2q = (S2 * np.asarray(b2, f)).astype(f)
    bhq = (S3 * bhc).astype(f)
    zero_bias = not (b1q.any() or b2q.any() or bhq.any())

    # weights pre-arranged to [partition, k_chunk * out]: A[p, c*out+o]
    # = Wq[c*128+p, o], so each partition's SBUF row is one contiguous run
    def warr(wq, chunks, out):
        return np.ascontiguousarray(
            wq.reshape(chunks, P, out).transpose(1, 0, 2).reshape(P, -1))

    # m-block-major variant: A[p, q, c, j] = Wq[c*128+p, q*bw+j], so each
    # m-quarter/half is its own contiguous per-partition run (own DMA)
    def warr_q(wq, chunks, nq, bw):
        return np.ascontiguousarray(
            wq.reshape(chunks, P, nq, bw).transpose(1, 2, 0, 3).reshape(P, -1))

    ball = np.zeros((P, 2 * HO + AO + 1), f)
    ball[:, 0:HO] = barr(b1q, HO)
    ball[:, HO:2 * HO] = barr(b2q, HO)
    ball[:, 2 * HO:2 * HO + AO] = barr(bhq, AO)
    ball[0:3, 2 * HO + AO] = bfin
    shared = dict(
        W1A=warr_q(q8(np.asarray(W1, f) * WS), KO, 4, 2 * P),
        W2A=warr_q(q8(np.asarray(W2, f) * WS), HO, 4, 2 * P),
        WhA=warr(q8(Wh * WS), HO, 2 * H),
        WfinA=warr(q8(Wfin * WS), AO, FINW),
        ballA=np.ascontiguousarray(ball),
    )
    # obs pre-arranged per core to [p, t*KO*NTILE + k*NTILE + j]
    # = obs_q[core*BC + t*NTILE + j, k*128+p]
    obs_q = np.clip(np.asarray(obs, f), -240.0, 240.0).astype(NP_F8)
    in_maps = []
    for c in range(NCORES):
        m = dict(shared)
        blk = obs_q[c * BC:(c + 1) * BC]                 # [BC, OBS]
        m["obsA"] = np.ascontiguousarray(
            blk.reshape(NT, NTILE, KO, P).transpose(3, 0, 2, 1).reshape(P, -1))
        in_maps.append(m)
    return in_maps, zero_bias, bfin


def assemble_outputs(res, bfin):
    """res[c]["sig"] is the raw [3, BC] fin-PSUM logits; concat cores, apply
    descale + bias + sigmoid, transpose to batch-major and broadcast each
    head to the 9 agent columns on the host."""
    raw = np.concatenate([res[c]["sig"] for c in range(NCORES)], axis=1)
    z = np.float32(DSF) * raw + np.asarray(bfin, np.float32)[:, None]
    sig = 1.0 / (1.0 + np.exp(-z, dtype=np.float32))
    return tuple(
        np.ascontiguousarray(
            np.broadcast_to(sig[i][:, None], (B, N)).astype(np.float32))
        for i in range(3))


def kernel(**inputs):
    in_maps, zero_bias, bfin = prep_inputs(**inputs)
    nc = _get_nc(zero_bias)
    res = run_bass_kernel_spmd(nc, in_maps, list(range(NCORES))).results
    return assemble_outputs(res, bfin)
